# revision 25
# baseline (speedup 1.0000x reference)
"""Trainium2 Bass kernel for nn_CrossAttention (B=16, Sq=4096, Skv=77, E=1024, H=16, D=64).

Sharding: data-parallel over batch — 16 batches / 8 cores = 2 batches per core.
Each core runs the full cross-attention for its 2 batches; no collectives.

Device dataflow (all-transposed, zero on-chip transposes):
  qT  [Eo, q]  = mm(lhsT=wq[Ei,Eo], rhs=xT[Ei,q])          (+bq via DVE per-partition add)
  kT  [Eo, kv] = mm(lhsT=wk'[Ci,Eo], rhs=yT[Ci,kv])        (wk' = wk/8: attn scale folded)
  v   [kv, Eo] = mm(lhsT=yT[Ci,kv], rhs=wv[Ci,Eo])         (bv folded into bo' on host)
  per head pair p (head 2p on partitions 0:64, head 2p+1 on 64:128):
    scoresT both heads -> one 2-bank PSUM tile [128, 2*QB]
      (row-groups 0:64 / 64:128 -> the two MMs run concurrently in the PE array)
    one ACT Exp over the merged [77, 2*QB] -> expT pair tile (bf16)
    avT h0 -> po[0:64], h1 -> po[64:128]  (col-groups -> concurrent)
    softmax denom z (ones-matmul, row-replicated) reuses scz bank A (col-groups)
    oinT = po * recip(z)  (DVE fast-reciprocal + tensor_tensor)
  finalT [Eo, q] = mm(lhsT=wo[Ei,Eo], rhs=oinT[Ei,q]) + bo' (ACT Identity w/ bias)

Software pipeline (emission order == engine priority order):
  item n = (batch, q-block). During item n's attention pairs we interleave the
  Q-projection groups of item n+1 (keeps PE busy through the exp/recip chains),
  and the O-projection of item n-1 runs after (its inputs long since ready).
  wv / wo weight DMAs are gated behind tiny reads depending on wk / wq so the
  startup-critical DMAs (wk, yT, wq, xT0) get the full HBM bandwidth first.

PSUM budget (8 banks): qf pool 2 + scz pool 2x2 + av pool 2 = 8.
"""

import os
import numpy as np
import ml_dtypes

import concourse.bass as bass
import concourse.mybir as mybir
from concourse import bacc
from concourse.tile import TileContext
from concourse import bass_utils

BF16 = mybir.dt.bfloat16
F32 = mybir.dt.float32
FP8 = mybir.dt.float8e4

# Q-projection precision knob: how many of the 4 K-chunk pairs run as
# fp8 DoubleRow (2x matmul rate); the rest run bf16. 4 = fully fp8.
FP8_PAIRS = 4
REM_EI = 8 - 2 * FP8_PAIRS  # bf16 contraction chunks remaining

# Problem shapes (hardcoded per contract)
B, SQ, SKV = 16, 4096, 77
E, C = 1024, 768
H, D = 16, 64
N_CORES = 8
B_PER_CORE = B // N_CORES  # 2

QB = 512                      # q rows per block
N_QB = SQ // QB               # 8 blocks per batch
EI_E = E // 128               # 8 contraction chunks over E
EI_C = C // 128               # 6 contraction chunks over C
EC = E // 128                 # 8 output chunks over E
PAIRS = H // 2                # 8 head pairs


def _build_program():
    nc = bacc.Bacc("TRN2", target_bir_lowering=False, debug=False)

    xT8 = nc.dram_tensor("xT8", [B_PER_CORE, E, SQ], FP8, kind="ExternalInput").ap()
    if REM_EI:
        xTb = nc.dram_tensor("xTb", [B_PER_CORE, 128 * REM_EI, SQ], BF16, kind="ExternalInput").ap()
        wqb = nc.dram_tensor("wqb", [128 * REM_EI, E], BF16, kind="ExternalInput").ap()
    yT = nc.dram_tensor("yT", [B_PER_CORE, C, SKV], BF16, kind="ExternalInput").ap()
    wq = nc.dram_tensor("wq", [E, E], FP8, kind="ExternalInput").ap()
    wk = nc.dram_tensor("wk", [C, E], BF16, kind="ExternalInput").ap()  # pre-scaled by 1/8
    wv = nc.dram_tensor("wv", [C, E], BF16, kind="ExternalInput").ap()
    wo = nc.dram_tensor("wo", [E, E], BF16, kind="ExternalInput").ap()
    # biases arrive pre-arranged [128, EC] so the DMA is a cheap contiguous
    # pattern (a flat [E] f32 with "(c p) -> p c" costs ~3us of descriptor
    # generation on the Sync engine, right on the startup critical path)
    bq = nc.dram_tensor("bq", [128, EC], F32, kind="ExternalInput").ap()
    bk = nc.dram_tensor("bk", [128, EC], F32, kind="ExternalInput").ap()  # pre-scaled by 1/8
    bo = nc.dram_tensor("bo", [128, EC], F32, kind="ExternalInput").ap()  # bo + bv@wo
    outT = nc.dram_tensor("outT", [B_PER_CORE, E, SQ], F32, kind="ExternalOutput").ap()

    with TileContext(nc) as tc:
        with (
            tc.tile_pool(name="const", bufs=1) as const,
            tc.tile_pool(name="batch", bufs=2) as batch,
            tc.tile_pool(name="xtiles", bufs=2) as xtiles,
            tc.tile_pool(name="qtiles", bufs=2) as qtiles,
            tc.tile_pool(name="exps", bufs=3) as exps,
            tc.tile_pool(name="rzs", bufs=2) as rzs,
            tc.tile_pool(name="oins", bufs=2) as oins,
            tc.tile_pool(name="outs", bufs=3) as outs,
            tc.tile_pool(name="ps_qf", bufs=2, space="PSUM") as ps_qf,
            tc.tile_pool(name="ps_scz", bufs=2, space="PSUM") as ps_scz,
            tc.tile_pool(name="ps_av", bufs=2, space="PSUM") as ps_av,
        ):
            # ---- resident weights/constants. wk/wq first: they gate the
            # startup-critical path; wv/wo DMAs are deferred via read-gates
            # below so they don't steal HBM bandwidth from wk/wq/xT0.
            gate = const.tile([1, 4], BF16)
            wk_sb = const.tile([128, EI_C, E], BF16)
            nc.sync.dma_start(wk_sb, wk.rearrange("(o p) n -> p o n", p=128))
            yT0_sb = batch.tile([128, EI_C, SKV], BF16, tag="yT")
            nc.sync.dma_start(yT0_sb, yT[0].rearrange("(o p) k -> p o k", p=128))
            # Big weight DMAs are staged ([wk,yT,x8_0,biases] -> wq -> wv -> wo)
            # via tiny read-gates so each stage gets the full HBM bandwidth and
            # compute starts as soon as its own stage has landed. Gated DMA
            # descriptors are emitted after the ungated ones: the Sync engine
            # stalls at a gated descriptor, so order matters.
            wq_sb = const.tile([128, EI_E, E], FP8)
            if REM_EI:
                wqb_sb = const.tile([128, REM_EI, E], BF16)
            bq_sb = const.tile([128, EC], F32)
            nc.sync.dma_start(bq_sb, bq)
            bk_sb = const.tile([128, EC], F32)
            nc.sync.dma_start(bk_sb, bk)
            bo_sb = const.tile([128, EC], F32)
            nc.sync.dma_start(bo_sb, bo)
            ones_blk = const.tile([SKV, 64], BF16)
            nc.vector.memset(ones_blk, 1.0)

            wv_sb = const.tile([128, EI_C, E], BF16)
            wo_sb = const.tile([128, EI_E, E], BF16)

            items = [(b, qb) for b in range(B_PER_CORE) for qb in range(N_QB)]
            kT_v = {}   # b -> (kT_sb, v_sb)
            qx = {}     # item -> qT_sb

            yT_tiles = {}

            def emit_kT_proj(b):
                if b == 0:
                    yT_sb = yT0_sb
                else:
                    yT_sb = batch.tile([128, EI_C, SKV], BF16, tag="yT")
                    nc.sync.dma_start(yT_sb, yT[b].rearrange("(o p) k -> p o k", p=128))
                yT_tiles[b] = yT_sb
                kT_sb = batch.tile([128, EC, SKV], BF16, tag="kT")
                for ec in range(EC):
                    pk = ps_qf.tile([128, QB], F32, tag="qf")
                    for ei in range(EI_C):
                        nc.tensor.matmul(
                            pk[:, :SKV],
                            wk_sb[:, ei, ec * 128:(ec + 1) * 128],
                            yT_sb[:, ei, :],
                            start=(ei == 0), stop=(ei == EI_C - 1),
                        )
                    nc.vector.tensor_scalar_add(kT_sb[:, ec, :], pk[:, :SKV], bk_sb[:, ec:ec + 1])
                return kT_sb

            def emit_v_proj(b, kT_sb):
                yT_sb = yT_tiles[b]
                v_sb = batch.tile([SKV, H, D], BF16, tag="v")
                for half in range(2):
                    pv = ps_qf.tile([128, QB], F32, tag="qf")
                    for ei in range(EI_C):
                        nc.tensor.matmul(
                            pv[:SKV, :],
                            yT_sb[:, ei, :],
                            wv_sb[:, ei, half * 512:(half + 1) * 512],
                            start=(ei == 0), stop=(ei == EI_C - 1),
                        )
                    nc.vector.tensor_copy(v_sb[:, half * 8:(half + 1) * 8, :], pv[:SKV, :].rearrange("p (h d) -> p h d", d=D))
                kT_v[b] = (kT_sb, v_sb)

            def emit_kv_proj(b):
                emit_v_proj(b, emit_kT_proj(b))

            def start_qproj(item):
                b, qb = item
                x8_sb = xtiles.tile([128, 2 * FP8_PAIRS, QB], FP8, tag="xT8")
                nc.sync.dma_start(
                    x8_sb,
                    xT8[b, 0:256 * FP8_PAIRS, qb * QB:(qb + 1) * QB].rearrange("(o p) q -> p o q", p=128),
                )
                xb_sb = None
                if REM_EI:
                    xb_sb = xtiles.tile([128, REM_EI, QB], BF16, tag="xTb")
                    nc.sync.dma_start(
                        xb_sb,
                        xTb[b, :, qb * QB:(qb + 1) * QB].rearrange("(o p) q -> p o q", p=128),
                    )
                qT_sb = qtiles.tile([128, EC, QB], BF16, tag="qT")
                qx[item] = (qT_sb, x8_sb, xb_sb)

            def emit_qproj_group(item, ec):
                qT_sb, x8_sb, xb_sb = qx[item]
                pq = ps_qf.tile([128, QB], F32, tag="qf")
                for i in range(FP8_PAIRS):
                    nc.tensor.matmul(
                        pq,
                        wq_sb[:, 2 * i:2 * i + 2, ec * 128:(ec + 1) * 128],
                        x8_sb[:, 2 * i:2 * i + 2, :],
                        start=(i == 0), stop=(i == FP8_PAIRS - 1 and REM_EI == 0),
                        perf_mode=mybir.MatmulPerfMode.DoubleRow,
                    )
                for j in range(REM_EI):
                    nc.tensor.matmul(
                        pq,
                        wqb_sb[:, j, ec * 128:(ec + 1) * 128],
                        xb_sb[:, j, :],
                        start=(FP8_PAIRS == 0 and j == 0), stop=(j == REM_EI - 1),
                    )
                nc.vector.tensor_scalar_add(qT_sb[:, ec, :], pq, bq_sb[:, ec:ec + 1])

            def emit_oproj(item, oinT_sb):
                b, qb = item
                for ec in range(EC):
                    pf = ps_qf.tile([128, QB], F32, tag="qf")
                    for p in range(PAIRS):
                        nc.tensor.matmul(
                            pf,
                            wo_sb[:, p, ec * 128:(ec + 1) * 128],
                            oinT_sb[:, p, :],
                            start=(p == 0), stop=(p == PAIRS - 1),
                        )
                    o_sb = outs.tile([128, QB], F32, tag="out")
                    nc.scalar.activation(
                        o_sb, pf, mybir.ActivationFunctionType.Identity,
                        bias=bo_sb[:, ec:ec + 1],
                    )
                    nc.sync.dma_start(outT[b, ec * 128:(ec + 1) * 128, qb * QB:(qb + 1) * QB], o_sb)

            # ---- prologue: kv(0) + full qproj(item 0) ----
            kT0_sb = emit_kT_proj(0)
            start_qproj(items[0])
            # wq DMA: gated on the last bias DMA, emitted after x8(0,0)'s
            # descriptor so the Sync stall doesn't delay it.
            nc.vector.tensor_tensor(gate[0:1, 0:1], wq_sb[0:1, 0, 0:1], bo_sb[0:1, 0:1], mybir.AluOpType.mult)
            nc.sync.dma_start(wq_sb, wq.rearrange("(o p) n -> p o n", p=128))
            if REM_EI:
                nc.sync.dma_start(wqb_sb, wqb.rearrange("(o p) n -> p o n", p=128))
            # wv waits until wq has landed; v-proj(0) is emitted after this
            nc.vector.tensor_tensor(gate[0:1, 2:3], wv_sb[0:1, 0, 0:1], wq_sb[0:1, 0, 0:1], mybir.AluOpType.mult)
            nc.sync.dma_start(wv_sb, wv.rearrange("(o p) n -> p o n", p=128))
            emit_v_proj(0, kT0_sb)
            emit_qproj_group(items[0], 0)
            # wo DMA deferred until the first q-projection output lands, so
            # the startup-critical DMAs (wk/wq/xT0) get the bandwidth first.
            qT0 = qx[items[0]][0]
            nc.vector.tensor_tensor(gate[0:1, 3:4], wo_sb[0:1, 0, 0:1], qT0[0:1, 0, 0:1], mybir.AluOpType.mult)
            nc.sync.dma_start(wo_sb, wo.rearrange("(o p) n -> p o n", p=128))
            for ec in range(1, EC):
                emit_qproj_group(items[0], ec)

            prev_oin = {}  # item -> oinT tile (consumed by the delayed oproj)

            for n, cur in enumerate(items):
                nxt = items[n + 1] if n + 1 < len(items) else None
                b, qb = cur
                if qb == 0 and b > 0:
                    emit_kv_proj(b)
                if nxt is not None:
                    start_qproj(nxt)
                kT_sb, v_sb = kT_v[b]
                qT_sb = qx[cur][0]

                oinT_sb = oins.tile([128, PAIRS, QB], BF16, tag="oinT")
                pend = []  # (p, scz, e, po) awaiting av/z/recip/mult

                def flush_pair():
                    p, scz, e, po = pend.pop(0)
                    # attn@v both heads (col-groups 0 / 64 -> concurrent)
                    nc.tensor.matmul(po[0:64, :], v_sb[:, 2 * p, :], e[:, 0:QB], start=True, stop=True)
                    nc.tensor.matmul(po[64:128, :], v_sb[:, 2 * p + 1, :], e[:, QB:2 * QB], start=True, stop=True)
                    # softmax denominators reuse scz bank A (WAR after the Exp)
                    nc.tensor.matmul(scz[0:64, 0:QB], ones_blk, e[:, 0:QB], start=True, stop=True)
                    nc.tensor.matmul(scz[64:128, 0:QB], ones_blk, e[:, QB:2 * QB], start=True, stop=True)
                    rz = rzs.tile([128, QB], F32, tag="rz")
                    nc.vector.reciprocal_approx_fast(rz, scz[:, 0:QB])
                    nc.vector.tensor_tensor(oinT_sb[:, p, :], po, rz, mybir.AluOpType.mult)

                for p in range(PAIRS):
                    # both heads' scores into one 2-bank psum tile
                    # (row-groups 0:64 / 64:128 -> concurrent)
                    scz = ps_scz.tile([128, 2 * QB], F32, tag="scz")
                    nc.tensor.matmul(
                        scz[:SKV, 0:QB], kT_sb[0:64, p, :], qT_sb[0:64, p, :],
                        start=True, stop=True,
                    )
                    nc.tensor.matmul(
                        scz[:SKV, QB:2 * QB], kT_sb[64:128, p, :], qT_sb[64:128, p, :],
                        start=True, stop=True,
                    )
                    # single Exp over the merged pair (scores tiny ~N(0,0.33):
                    # no max-subtraction needed)
                    e = exps.tile([SKV, 2 * QB], BF16, tag="expT")
                    nc.scalar.activation(e, scz[:SKV, :], mybir.ActivationFunctionType.Exp)
                    po = ps_av.tile([128, QB], F32, tag="av")
                    pend.append((p, scz, e, po))
                    if len(pend) > 1:
                        flush_pair()
                    # interleave next item's q-projection: PE work that covers
                    # the exp -> av -> recip dependency chain of this pair
                    if nxt is not None:
                        emit_qproj_group(nxt, p)
                while pend:
                    flush_pair()
                prev_oin[cur] = oinT_sb

                if n > 0:
                    prv = items[n - 1]
                    emit_oproj(prv, prev_oin.pop(prv))

            last = items[-1]
            emit_oproj(last, prev_oin.pop(last))

    nc.compile()
    return nc


_CACHED = {}


def _get_program():
    if "nc" not in _CACHED:
        _CACHED["nc"] = _build_program()
    return _CACHED["nc"]


def kernel(**inputs):
    x = np.asarray(inputs["x"], dtype=np.float32)
    y = np.asarray(inputs["y"], dtype=np.float32)
    wq = np.asarray(inputs["wq"], dtype=np.float32)
    bq = np.asarray(inputs["bq"], dtype=np.float32)
    wk = np.asarray(inputs["wk"], dtype=np.float32)
    bk = np.asarray(inputs["bk"], dtype=np.float32)
    wv = np.asarray(inputs["wv"], dtype=np.float32)
    bv = np.asarray(inputs["bv"], dtype=np.float32)
    wo = np.asarray(inputs["wo"], dtype=np.float32)
    bo = np.asarray(inputs["bo"], dtype=np.float32)

    bf = ml_dtypes.bfloat16
    f8 = ml_dtypes.float8_e4m3
    scale = 1.0 / np.sqrt(np.float32(D))

    # host-side prep: transpose activations, cast, fold scale & bv
    xT = x.transpose(0, 2, 1)                                           # [B, E, Sq]
    xT8 = np.ascontiguousarray(xT.astype(f8))
    yT = np.ascontiguousarray(y.astype(bf).transpose(0, 2, 1))          # [B, C, Skv]
    wq_8 = np.ascontiguousarray(wq.astype(f8))
    wk_b = np.ascontiguousarray((wk * scale).astype(bf))
    wv_b = np.ascontiguousarray(wv.astype(bf))
    wo_b = np.ascontiguousarray(wo.astype(bf))
    def barr(v):  # [E] -> [128, EC] partition-major layout
        return np.ascontiguousarray(v.astype(np.float32).reshape(EC, 128).T)
    bk_s = barr(bk * scale)
    bo_f = barr(bo + bv @ wo)
    bq_f = barr(bq)

    nc = _get_program()
    in_maps = []
    for c in range(N_CORES):
        m = {
            "xT8": np.ascontiguousarray(xT8[c * B_PER_CORE:(c + 1) * B_PER_CORE]),
            "yT": np.ascontiguousarray(yT[c * B_PER_CORE:(c + 1) * B_PER_CORE]),
            "wq": wq_8, "wk": wk_b, "wv": wv_b, "wo": wo_b,
            "bq": bq_f, "bk": bk_s, "bo": bo_f,
        }
        if REM_EI:
            m["xTb"] = np.ascontiguousarray(
                xT[c * B_PER_CORE:(c + 1) * B_PER_CORE, 256 * FP8_PAIRS:].astype(bf))
            m["wqb"] = np.ascontiguousarray(wq[256 * FP8_PAIRS:].astype(bf))
        in_maps.append(m)

    _CACHED["in_maps"] = in_maps
    res = bass_utils.run_bass_kernel_spmd(
        nc, in_maps, core_ids=list(range(N_CORES)),
    )
    _CACHED["last_results"] = res

    out = np.empty((B, SQ, E), dtype=np.float32)
    for c in range(N_CORES):
        oT = res.results[c]["outT"]  # [B_PER_CORE, E, SQ] f32
        out[c * B_PER_CORE:(c + 1) * B_PER_CORE] = oT.transpose(0, 2, 1)
    return out


# revision 26
# speedup vs baseline: 1.0139x; 1.0139x over previous
"""Trainium2 Bass kernel for nn_CrossAttention (B=16, Sq=4096, Skv=77, E=1024, H=16, D=64).

Sharding: data-parallel over batch — 16 batches / 8 cores = 2 batches per core.
Each core runs the full cross-attention for its 2 batches; no collectives.

Device dataflow (all-transposed, zero on-chip transposes):
  qT  [Eo, q]  = mm(lhsT=wq[Ei,Eo], rhs=xT[Ei,q])          (+bq via DVE per-partition add)
  kT  [Eo, kv] = mm(lhsT=wk'[Ci,Eo], rhs=yT[Ci,kv])        (wk' = wk/8: attn scale folded)
  v   [kv, Eo] = mm(lhsT=yT[Ci,kv], rhs=wv[Ci,Eo])         (bv folded into bo' on host)
  per head pair p (head 2p on partitions 0:64, head 2p+1 on 64:128):
    scoresT both heads -> one 2-bank PSUM tile [128, 2*QB]
      (row-groups 0:64 / 64:128 -> the two MMs run concurrently in the PE array)
    one ACT Exp over the merged [77, 2*QB] -> expT pair tile (bf16)
    avT h0 -> po[0:64], h1 -> po[64:128]  (col-groups -> concurrent)
    softmax denom z (ones-matmul, row-replicated) reuses scz bank A (col-groups)
    oinT = po * recip(z)  (DVE fast-reciprocal + tensor_tensor)
  finalT [Eo, q] = mm(lhsT=wo[Ei,Eo], rhs=oinT[Ei,q]) + bo' (ACT Identity w/ bias)

Software pipeline (emission order == engine priority order):
  item n = (batch, q-block). During item n's attention pairs we interleave the
  Q-projection groups of item n+1 (keeps PE busy through the exp/recip chains),
  and the O-projection of item n-1 runs after (its inputs long since ready).
  wv / wo weight DMAs are gated behind tiny reads depending on wk / wq so the
  startup-critical DMAs (wk, yT, wq, xT0) get the full HBM bandwidth first.

PSUM budget (8 banks): qf pool 2 + scz pool 2x2 + av pool 2 = 8.
"""

import os
import numpy as np
import ml_dtypes

import concourse.bass as bass
import concourse.mybir as mybir
from concourse import bacc
from concourse.tile import TileContext
from concourse import bass_utils

BF16 = mybir.dt.bfloat16
F32 = mybir.dt.float32
FP8 = mybir.dt.float8e4

# Q-projection precision knob: how many of the 4 K-chunk pairs run as
# fp8 DoubleRow (2x matmul rate); the rest run bf16. 4 = fully fp8.
FP8_PAIRS = 4
REM_EI = 8 - 2 * FP8_PAIRS  # bf16 contraction chunks remaining

# Problem shapes (hardcoded per contract)
B, SQ, SKV = 16, 4096, 77
E, C = 1024, 768
H, D = 16, 64
N_CORES = 8
B_PER_CORE = B // N_CORES  # 2

QB = 512                      # q rows per block
N_QB = SQ // QB               # 8 blocks per batch
EI_E = E // 128               # 8 contraction chunks over E
EI_C = C // 128               # 6 contraction chunks over C
EC = E // 128                 # 8 output chunks over E
PAIRS = H // 2                # 8 head pairs


def _build_program():
    nc = bacc.Bacc("TRN2", target_bir_lowering=False, debug=False)

    xT8 = nc.dram_tensor("xT8", [B_PER_CORE, E, SQ], FP8, kind="ExternalInput").ap()
    if REM_EI:
        xTb = nc.dram_tensor("xTb", [B_PER_CORE, 128 * REM_EI, SQ], BF16, kind="ExternalInput").ap()
        wqb = nc.dram_tensor("wqb", [128 * REM_EI, E], BF16, kind="ExternalInput").ap()
    yT = nc.dram_tensor("yT", [B_PER_CORE, C, SKV], BF16, kind="ExternalInput").ap()
    wq = nc.dram_tensor("wq", [E, E], FP8, kind="ExternalInput").ap()
    wk = nc.dram_tensor("wk", [C, E], BF16, kind="ExternalInput").ap()  # pre-scaled by 1/8
    wv = nc.dram_tensor("wv", [C, E], BF16, kind="ExternalInput").ap()
    wo = nc.dram_tensor("wo", [E, E], BF16, kind="ExternalInput").ap()
    # biases arrive pre-arranged [128, EC] so the DMA is a cheap contiguous
    # pattern (a flat [E] f32 with "(c p) -> p c" costs ~3us of descriptor
    # generation on the Sync engine, right on the startup critical path)
    bq = nc.dram_tensor("bq", [128, EC], F32, kind="ExternalInput").ap()
    bk = nc.dram_tensor("bk", [128, EC], F32, kind="ExternalInput").ap()  # pre-scaled by 1/8
    bo = nc.dram_tensor("bo", [128, EC], F32, kind="ExternalInput").ap()  # bo + bv@wo
    outT = nc.dram_tensor("outT", [B_PER_CORE, E, SQ], F32, kind="ExternalOutput").ap()

    with TileContext(nc) as tc:
        with (
            tc.tile_pool(name="const", bufs=1) as const,
            tc.tile_pool(name="batch", bufs=2) as batch,
            tc.tile_pool(name="xtiles", bufs=2) as xtiles,
            tc.tile_pool(name="qtiles", bufs=2) as qtiles,
            tc.tile_pool(name="exps", bufs=3) as exps,
            tc.tile_pool(name="rzs", bufs=2) as rzs,
            tc.tile_pool(name="oins", bufs=2) as oins,
            tc.tile_pool(name="outs", bufs=3) as outs,
            tc.tile_pool(name="ps_qf", bufs=2, space="PSUM") as ps_qf,
            tc.tile_pool(name="ps_scz", bufs=2, space="PSUM") as ps_scz,
            tc.tile_pool(name="ps_av", bufs=2, space="PSUM") as ps_av,
        ):
            # ---- resident weights/constants. wk/wq first: they gate the
            # startup-critical path; wv/wo DMAs are deferred via read-gates
            # below so they don't steal HBM bandwidth from wk/wq/xT0.
            gate = const.tile([1, 4], BF16)
            wk_sb = const.tile([128, EI_C, E], BF16)
            nc.sync.dma_start(wk_sb, wk.rearrange("(o p) n -> p o n", p=128))
            yT0_sb = batch.tile([128, EI_C, SKV], BF16, tag="yT")
            nc.sync.dma_start(yT0_sb, yT[0].rearrange("(o p) k -> p o k", p=128))
            # Big weight DMAs are staged ([wk,yT,x8_0,biases] -> wq -> wv -> wo)
            # via tiny read-gates so each stage gets the full HBM bandwidth and
            # compute starts as soon as its own stage has landed. Gated DMA
            # descriptors are emitted after the ungated ones: the Sync engine
            # stalls at a gated descriptor, so order matters.
            wq_sb = const.tile([128, EI_E, E], FP8)
            if REM_EI:
                wqb_sb = const.tile([128, REM_EI, E], BF16)
            bq_sb = const.tile([128, EC], F32)
            nc.sync.dma_start(bq_sb, bq)
            bk_sb = const.tile([128, EC], F32)
            nc.sync.dma_start(bk_sb, bk)
            bo_sb = const.tile([128, EC], F32)
            nc.sync.dma_start(bo_sb, bo)
            ones_blk = const.tile([SKV, 64], BF16)
            nc.vector.memset(ones_blk, 1.0)

            wv_sb = const.tile([128, EI_C, E], BF16)
            wo_sb = const.tile([128, EI_E, E], BF16)

            items = [(b, qb) for b in range(B_PER_CORE) for qb in range(N_QB)]
            kT_v = {}   # b -> (kT_sb, v_sb)
            qx = {}     # item -> qT_sb

            yT_tiles = {}

            def emit_kT_proj(b):
                if b == 0:
                    yT_sb = yT0_sb
                else:
                    yT_sb = batch.tile([128, EI_C, SKV], BF16, tag="yT")
                    nc.sync.dma_start(yT_sb, yT[b].rearrange("(o p) k -> p o k", p=128))
                yT_tiles[b] = yT_sb
                kT_sb = batch.tile([128, EC, SKV], BF16, tag="kT")
                for ec in range(EC):
                    pk = ps_qf.tile([128, QB], F32, tag="qf")
                    for ei in range(EI_C):
                        nc.tensor.matmul(
                            pk[:, :SKV],
                            wk_sb[:, ei, ec * 128:(ec + 1) * 128],
                            yT_sb[:, ei, :],
                            start=(ei == 0), stop=(ei == EI_C - 1),
                        )
                    nc.vector.tensor_scalar_add(kT_sb[:, ec, :], pk[:, :SKV], bk_sb[:, ec:ec + 1])
                return kT_sb

            def emit_v_proj(b, kT_sb):
                yT_sb = yT_tiles[b]
                v_sb = batch.tile([SKV, H, D], BF16, tag="v")
                for half in range(2):
                    pv = ps_qf.tile([128, QB], F32, tag="qf")
                    for ei in range(EI_C):
                        nc.tensor.matmul(
                            pv[:SKV, :],
                            yT_sb[:, ei, :],
                            wv_sb[:, ei, half * 512:(half + 1) * 512],
                            start=(ei == 0), stop=(ei == EI_C - 1),
                        )
                    nc.vector.tensor_copy(v_sb[:, half * 8:(half + 1) * 8, :], pv[:SKV, :].rearrange("p (h d) -> p h d", d=D))
                kT_v[b] = (kT_sb, v_sb)

            def emit_kv_proj(b):
                emit_v_proj(b, emit_kT_proj(b))

            def start_qproj(item):
                b, qb = item
                x8_sb = xtiles.tile([128, 2 * FP8_PAIRS, QB], FP8, tag="xT8")
                nc.sync.dma_start(
                    x8_sb,
                    xT8[b, 0:256 * FP8_PAIRS, qb * QB:(qb + 1) * QB].rearrange("(o p) q -> p o q", p=128),
                )
                xb_sb = None
                if REM_EI:
                    xb_sb = xtiles.tile([128, REM_EI, QB], BF16, tag="xTb")
                    nc.sync.dma_start(
                        xb_sb,
                        xTb[b, :, qb * QB:(qb + 1) * QB].rearrange("(o p) q -> p o q", p=128),
                    )
                qT_sb = qtiles.tile([128, EC, QB], BF16, tag="qT")
                qx[item] = (qT_sb, x8_sb, xb_sb)

            def emit_qproj_group(item, ec):
                qT_sb, x8_sb, xb_sb = qx[item]
                pq = ps_qf.tile([128, QB], F32, tag="qf")
                for i in range(FP8_PAIRS):
                    nc.tensor.matmul(
                        pq,
                        wq_sb[:, 2 * i:2 * i + 2, ec * 128:(ec + 1) * 128],
                        x8_sb[:, 2 * i:2 * i + 2, :],
                        start=(i == 0), stop=(i == FP8_PAIRS - 1 and REM_EI == 0),
                        perf_mode=mybir.MatmulPerfMode.DoubleRow,
                    )
                for j in range(REM_EI):
                    nc.tensor.matmul(
                        pq,
                        wqb_sb[:, j, ec * 128:(ec + 1) * 128],
                        xb_sb[:, j, :],
                        start=(FP8_PAIRS == 0 and j == 0), stop=(j == REM_EI - 1),
                    )
                nc.vector.tensor_scalar_add(qT_sb[:, ec, :], pq, bq_sb[:, ec:ec + 1])

            def emit_oproj(item, oinT_sb):
                b, qb = item
                for ec in range(EC):
                    pf = ps_qf.tile([128, QB], F32, tag="qf")
                    for p in range(PAIRS):
                        nc.tensor.matmul(
                            pf,
                            wo_sb[:, p, ec * 128:(ec + 1) * 128],
                            oinT_sb[:, p, :],
                            start=(p == 0), stop=(p == PAIRS - 1),
                        )
                    o_sb = outs.tile([128, QB], F32, tag="out")
                    nc.scalar.activation(
                        o_sb, pf, mybir.ActivationFunctionType.Identity,
                        bias=bo_sb[:, ec:ec + 1],
                    )
                    nc.sync.dma_start(outT[b, ec * 128:(ec + 1) * 128, qb * QB:(qb + 1) * QB], o_sb)

            # ---- prologue: kv(0) + full qproj(item 0) ----
            kT0_sb = emit_kT_proj(0)
            start_qproj(items[0])
            # wq DMA: gated on the last bias DMA, emitted after x8(0,0)'s
            # descriptor so the Sync stall doesn't delay it.
            nc.vector.tensor_tensor(gate[0:1, 0:1], wq_sb[0:1, 0, 0:1], bo_sb[0:1, 0:1], mybir.AluOpType.mult)
            nc.sync.dma_start(wq_sb, wq.rearrange("(o p) n -> p o n", p=128))
            if REM_EI:
                nc.sync.dma_start(wqb_sb, wqb.rearrange("(o p) n -> p o n", p=128))
            # wv waits until wk has landed; v-proj(0) is emitted after this
            nc.vector.tensor_tensor(gate[0:1, 2:3], wv_sb[0:1, 0, 0:1], wk_sb[0:1, 0, 0:1], mybir.AluOpType.mult)
            nc.sync.dma_start(wv_sb, wv.rearrange("(o p) n -> p o n", p=128))
            emit_v_proj(0, kT0_sb)
            emit_qproj_group(items[0], 0)
            # wo DMA deferred until the first q-projection output lands, so
            # the startup-critical DMAs (wk/wq/xT0) get the bandwidth first.
            qT0 = qx[items[0]][0]
            nc.vector.tensor_tensor(gate[0:1, 3:4], wo_sb[0:1, 0, 0:1], qT0[0:1, 0, 0:1], mybir.AluOpType.mult)
            nc.sync.dma_start(wo_sb, wo.rearrange("(o p) n -> p o n", p=128))
            for ec in range(1, EC):
                emit_qproj_group(items[0], ec)

            prev_oin = {}  # item -> oinT tile (consumed by the delayed oproj)

            for n, cur in enumerate(items):
                nxt = items[n + 1] if n + 1 < len(items) else None
                b, qb = cur
                if qb == 0 and b > 0:
                    emit_kv_proj(b)
                if nxt is not None:
                    start_qproj(nxt)
                kT_sb, v_sb = kT_v[b]
                qT_sb = qx[cur][0]

                oinT_sb = oins.tile([128, PAIRS, QB], BF16, tag="oinT")
                pend = []  # (p, scz, e, po) awaiting av/z/recip/mult

                def flush_pair():
                    p, scz, e, po = pend.pop(0)
                    # attn@v both heads (col-groups 0 / 64 -> concurrent)
                    nc.tensor.matmul(po[0:64, :], v_sb[:, 2 * p, :], e[:, 0:QB], start=True, stop=True)
                    nc.tensor.matmul(po[64:128, :], v_sb[:, 2 * p + 1, :], e[:, QB:2 * QB], start=True, stop=True)
                    # softmax denominators reuse scz bank A (WAR after the Exp)
                    nc.tensor.matmul(scz[0:64, 0:QB], ones_blk, e[:, 0:QB], start=True, stop=True)
                    nc.tensor.matmul(scz[64:128, 0:QB], ones_blk, e[:, QB:2 * QB], start=True, stop=True)
                    rz = rzs.tile([128, QB], F32, tag="rz")
                    nc.vector.reciprocal_approx_fast(rz, scz[:, 0:QB])
                    nc.vector.tensor_tensor(oinT_sb[:, p, :], po, rz, mybir.AluOpType.mult)

                for p in range(PAIRS):
                    # both heads' scores into one 2-bank psum tile
                    # (row-groups 0:64 / 64:128 -> concurrent)
                    scz = ps_scz.tile([128, 2 * QB], F32, tag="scz")
                    nc.tensor.matmul(
                        scz[:SKV, 0:QB], kT_sb[0:64, p, :], qT_sb[0:64, p, :],
                        start=True, stop=True,
                    )
                    nc.tensor.matmul(
                        scz[:SKV, QB:2 * QB], kT_sb[64:128, p, :], qT_sb[64:128, p, :],
                        start=True, stop=True,
                    )
                    # single Exp over the merged pair (scores tiny ~N(0,0.33):
                    # no max-subtraction needed)
                    e = exps.tile([SKV, 2 * QB], BF16, tag="expT")
                    nc.scalar.activation(e, scz[:SKV, :], mybir.ActivationFunctionType.Exp)
                    po = ps_av.tile([128, QB], F32, tag="av")
                    pend.append((p, scz, e, po))
                    if len(pend) > 1:
                        flush_pair()
                    # interleave next item's q-projection: PE work that covers
                    # the exp -> av -> recip dependency chain of this pair
                    if nxt is not None:
                        emit_qproj_group(nxt, p)
                while pend:
                    flush_pair()
                prev_oin[cur] = oinT_sb

                if n > 0:
                    prv = items[n - 1]
                    emit_oproj(prv, prev_oin.pop(prv))

            last = items[-1]
            emit_oproj(last, prev_oin.pop(last))

    nc.compile()
    return nc


_CACHED = {}


def _get_program():
    if "nc" not in _CACHED:
        _CACHED["nc"] = _build_program()
    return _CACHED["nc"]


def kernel(**inputs):
    x = np.asarray(inputs["x"], dtype=np.float32)
    y = np.asarray(inputs["y"], dtype=np.float32)
    wq = np.asarray(inputs["wq"], dtype=np.float32)
    bq = np.asarray(inputs["bq"], dtype=np.float32)
    wk = np.asarray(inputs["wk"], dtype=np.float32)
    bk = np.asarray(inputs["bk"], dtype=np.float32)
    wv = np.asarray(inputs["wv"], dtype=np.float32)
    bv = np.asarray(inputs["bv"], dtype=np.float32)
    wo = np.asarray(inputs["wo"], dtype=np.float32)
    bo = np.asarray(inputs["bo"], dtype=np.float32)

    bf = ml_dtypes.bfloat16
    f8 = ml_dtypes.float8_e4m3
    scale = 1.0 / np.sqrt(np.float32(D))

    # host-side prep: transpose activations, cast, fold scale & bv
    xT = x.transpose(0, 2, 1)                                           # [B, E, Sq]
    xT8 = np.ascontiguousarray(xT.astype(f8))
    yT = np.ascontiguousarray(y.astype(bf).transpose(0, 2, 1))          # [B, C, Skv]
    wq_8 = np.ascontiguousarray(wq.astype(f8))
    wk_b = np.ascontiguousarray((wk * scale).astype(bf))
    wv_b = np.ascontiguousarray(wv.astype(bf))
    wo_b = np.ascontiguousarray(wo.astype(bf))
    def barr(v):  # [E] -> [128, EC] partition-major layout
        return np.ascontiguousarray(v.astype(np.float32).reshape(EC, 128).T)
    bk_s = barr(bk * scale)
    bo_f = barr(bo + bv @ wo)
    bq_f = barr(bq)

    nc = _get_program()
    in_maps = []
    for c in range(N_CORES):
        m = {
            "xT8": np.ascontiguousarray(xT8[c * B_PER_CORE:(c + 1) * B_PER_CORE]),
            "yT": np.ascontiguousarray(yT[c * B_PER_CORE:(c + 1) * B_PER_CORE]),
            "wq": wq_8, "wk": wk_b, "wv": wv_b, "wo": wo_b,
            "bq": bq_f, "bk": bk_s, "bo": bo_f,
        }
        if REM_EI:
            m["xTb"] = np.ascontiguousarray(
                xT[c * B_PER_CORE:(c + 1) * B_PER_CORE, 256 * FP8_PAIRS:].astype(bf))
            m["wqb"] = np.ascontiguousarray(wq[256 * FP8_PAIRS:].astype(bf))
        in_maps.append(m)

    _CACHED["in_maps"] = in_maps
    res = bass_utils.run_bass_kernel_spmd(
        nc, in_maps, core_ids=list(range(N_CORES)),
    )
    _CACHED["last_results"] = res

    out = np.empty((B, SQ, E), dtype=np.float32)
    for c in range(N_CORES):
        oT = res.results[c]["outT"]  # [B_PER_CORE, E, SQ] f32
        out[c * B_PER_CORE:(c + 1) * B_PER_CORE] = oT.transpose(0, 2, 1)
    return out
